# revision 4
# baseline (speedup 1.0000x reference)
"""Trainium2 Bass kernel for MultiHeadSelfAttention with RoPE (bf16 path).

Problem: x[2, 2048, 1024] @ W_qkv[1024, 3072] -> rope(q,k) -> softmax(q k^T/8) v
         -> out @ W_out[1024, 1024].

Sharding (8 cores): batch (2-way) x head-group (4-way, 4 heads each).
Each core computes a partial output [2048, 1024] = attnout_heads @ W_out_rows;
host sums the 4 head-group partials per batch.

All matmuls run in bf16 (inputs pre-cast on host; FWL hides the weight loads),
accumulating in fp32 PSUM. Elementwise work is bf16 end-to-end so the DVE gets
its 2x packed mode. Measured-rel-err budget is 2e-2; bf16 lands ~6e-3.

On-core dataflow is fully "transposed" so the PE never needs a transpose:
  qT,kT[c, s] = sum_e W[e, c] * xT[e, s]   (lhsT = W slice, rhs = xT)
  rot = Mswap @ qT (PE), q' = qT*cos + rot*sin_signed (DVE)
  scores[sk, sq] per head via K=128 packing: [kT_A|kT_B] against zero-padded
  q ([q_A|0] / [0|q_B]); both heads' 512-col scores land in one [128, 1024]
  PSUM tile so a single ScalarE exp (scale=1/8 folded) serves the pair.
  attnT[sk, sq] -> oT[d, sq] += [v|1]^T attn (ones column gives the softmax
  denominator in row 64 for free); normalize via ones-outer-product broadcast
  + reciprocal + multiply; out_partial[s, e] = att_oT.T @ W_out_rows.
"""

import sys

if "/opt/trn_rl_repo" not in sys.path:
    sys.path.insert(0, "/opt/trn_rl_repo")

import numpy as np

B, S, E = 2, 2048, 1024
ATT = 1024
H = 16
D = 64
HG = 4            # head groups (cores per batch)
HPG = H // HG     # heads per core = 4
PAIRS = HPG // 2  # head pairs per core = 2
ROPE_THETA = 10000.0
N_CORES = 8

CH = 512              # sq chunk for the attention inner loop
N_CH = S // CH        # 4 chunks
N_SK = S // 128       # 16 sk tiles
EK = E // 128         # 8 contraction tiles over embedding dim
NSC = S // 512        # 4 s-chunks for xT staging

_BUILT = {}


def _build_program():
    import concourse.bacc as bacc
    import concourse.tile as tile
    import concourse.mybir as mybir

    f32 = mybir.dt.float32
    bf16 = mybir.dt.bfloat16
    AF = mybir.ActivationFunctionType

    nc = bacc.Bacc(
        "TRN2",
        target_bir_lowering=False,
        debug=False,
        enable_asserts=False,
        num_devices=N_CORES,
    )

    xT = nc.dram_tensor("xT", [E, S], bf16, kind="ExternalInput").ap()
    w_qk = nc.dram_tensor("w_qk", [E, 2 * HPG * D], bf16, kind="ExternalInput").ap()
    w_v = nc.dram_tensor("w_v", [E, HPG * D], bf16, kind="ExternalInput").ap()
    w_o = nc.dram_tensor("w_o", [HPG * D, E], bf16, kind="ExternalInput").ap()
    cos_t = nc.dram_tensor("cos_t", [128, S], bf16, kind="ExternalInput").ap()
    sin_t = nc.dram_tensor("sin_t", [128, S], bf16, kind="ExternalInput").ap()
    mswap = nc.dram_tensor("mswap", [128, 128], bf16, kind="ExternalInput").ap()
    out = nc.dram_tensor("out", [S, E], f32, kind="ExternalOutput").ap()

    with tile.TileContext(nc) as tc:
        with (
            tc.tile_pool(name="const", bufs=1) as constp,
            tc.tile_pool(name="qkT", bufs=1) as qkTp,
            tc.tile_pool(name="vsb", bufs=1) as vp,
            tc.tile_pool(name="attnout", bufs=1) as aop,
            tc.tile_pool(name="wo", bufs=1) as wop,
        ):
            msw_sb = constp.tile([128, 128], bf16, tag="msw")
            # ones row at partition 64 so its base matches the denominator
            # rhs operand oX[64:65] of the broadcast matmuls
            onesrow = constp.tile([65, 64], bf16, tag="onesrow")
            nc.gpsimd.memset(onesrow[64:65, :], 1.0)
            ones_bf = constp.tile([128, N_SK], bf16, tag="ones_bf")
            nc.gpsimd.memset(ones_bf[:], 1.0)
            # ACT warmup: get the exp table-set load off the critical path
            warm = constp.tile([128, 16], bf16, tag="warm")
            nc.scalar.activation(warm[:], ones_bf[:, 0:16], AF.Exp, scale=0.125)

            # k' per pair: [128, S] (rows 0:64 head A dims, 64:128 head B).
            # q' per pair split into two zero-padded [128, S] tensors so the
            # scores matmuls contract over the full K=128 (2-head packing):
            # qzlo = [q'_A | 0], qzhi = [0 | q'_B].
            qzlo = [qkTp.tile([128, S], bf16, tag=f"qzlo{g}", name=f"qzlo{g}") for g in range(PAIRS)]
            qzhi = [qkTp.tile([128, S], bf16, tag=f"qzhi{g}", name=f"qzhi{g}") for g in range(PAIRS)]
            kT = [qkTp.tile([128, S], bf16, tag=f"kT{g}", name=f"kT{g}") for g in range(PAIRS)]
            for g in range(PAIRS):
                nc.gpsimd.memset(qzlo[g][64:128, :], 0.0)
                nc.gpsimd.memset(qzhi[g][0:64, :], 0.0)
            # v natural + aug ones column, 4 heads: head h occupies cols
            # [65h, 65h+64) = v, col 65h+64 = ones (softmax-denominator row)
            v_c = vp.tile([128, N_SK, 4 * 65], bf16, tag="vc", name="vc")
            for h in range(4):
                nc.vector.tensor_copy(v_c[:, :, 65 * h + 64], ones_bf[:])
            # normalized attention output per pair [128 (pair dims), S]
            att_o = [aop.tile([128, S], bf16, tag=f"ao{g}", name=f"ao{g}") for g in range(PAIRS)]
            # W_out rows per pair
            wo_sb = [wop.tile([128, E], bf16, tag=f"wo{g}", name=f"wo{g}") for g in range(PAIRS)]

            with (
                tc.tile_pool(name="xt", bufs=EK * NSC) as xtp,
                tc.tile_pool(name="wqk", bufs=EK) as wqkp,
                tc.tile_pool(name="wv", bufs=EK) as wvp,
                tc.tile_pool(name="ropes", bufs=3) as ropep,
                tc.tile_pool(name="trig", bufs=1) as trigp,
                tc.tile_pool(name="projps", bufs=3, space="PSUM") as pjp,
                tc.tile_pool(name="rotps", bufs=2, space="PSUM") as rtp,
                tc.tile_pool(name="vps", bufs=2, space="PSUM") as vpp,
            ):
                cos_sb = trigp.tile([128, S], bf16, tag="cos")
                sin_sb = trigp.tile([128, S], bf16, tag="sin")
                # DMA order = need order: qk weights, then x s-chunk 0, ...
                wqk_sb = []
                for e in range(EK):
                    t = wqkp.tile([128, 2 * HPG * D], bf16, tag="wqk")
                    nc.sync.dma_start(t[:], w_qk[128 * e : 128 * (e + 1), :])
                    wqk_sb.append(t)
                # xt_sb[e][c] = xT[128e:128e+128, 512c:512c+512]
                xt_sb = [[None] * NSC for _ in range(EK)]
                for c in range(NSC):
                    for e in range(EK):
                        t = xtp.tile([128, 512], bf16, tag="xt")
                        nc.sync.dma_start(
                            t[:], xT[128 * e : 128 * (e + 1), 512 * c : 512 * (c + 1)]
                        )
                        xt_sb[e][c] = t
                nc.sync.dma_start(msw_sb[:], mswap[:])
                nc.sync.dma_start(cos_sb[:], cos_t[:])
                nc.sync.dma_start(sin_sb[:], sin_t[:])
                wv_sb = []
                for e in range(EK):
                    t = wvp.tile([128, HPG * D], bf16, tag="wv")
                    nc.sync.dma_start(t[:], w_v[128 * e : 128 * (e + 1), :])
                    wv_sb.append(t)
                for g in range(PAIRS):
                    nc.sync.dma_start(wo_sb[g][:], w_o[128 * g : 128 * (g + 1), :])

                rope_pend = []

                def rope_tail():
                    (g_, dest, sl, raw) = rope_pend.pop(0)
                    rp = rtp.tile([128, 512], f32, tag="rot")
                    nc.tensor.matmul(rp[:], msw_sb[:], raw[:], start=True, stop=True)
                    # keep DVE ops same-dtype bf16 so 2x packed mode engages
                    rps = ropep.tile([128, 512], bf16, tag="rps")
                    nc.scalar.copy(rps[:], rp[:])
                    t2 = ropep.tile([128, 512], bf16, tag="t2")
                    nc.vector.tensor_mul(t2[:], raw[:], cos_sb[:, sl])
                    t1 = ropep.tile([128, 512], bf16, tag="t1")
                    nc.vector.tensor_mul(t1[:], rps[:], sin_sb[:, sl])
                    if dest is None:
                        nc.vector.tensor_add(qzlo[g_][0:64, sl], t1[0:64, :], t2[0:64, :])
                        nc.vector.tensor_add(qzhi[g_][64:128, sl], t1[64:128, :], t2[64:128, :])
                    else:
                        nc.vector.tensor_add(dest[:, sl], t1[:], t2[:])

                def proj_qk(g):
                    # qT / kT projection + rope, chunked over s
                    for ti, dest in ((0, None), (1, kT[g])):
                        coff = ti * HPG * D + 128 * g
                        for c in range(NSC):
                            sl = slice(512 * c, 512 * (c + 1))
                            pp = pjp.tile([128, 512], f32, tag="pj")
                            for e in range(EK):
                                nc.tensor.matmul(
                                    pp[:],
                                    wqk_sb[e][:, coff : coff + 128],
                                    xt_sb[e][c][:],
                                    start=(e == 0),
                                    stop=(e == EK - 1),
                                )
                            raw = ropep.tile([128, 512], bf16, tag="raw")
                            nc.scalar.copy(raw[:], pp[:])
                            rope_pend.append((g, dest, sl, raw))
                            if len(rope_pend) > 1:
                                rope_tail()

                def proj_v(st):
                    vp_ps = vpp.tile([128, 2 * 128], f32, tag="vps")
                    for e in range(EK):
                        nc.tensor.matmul(
                            vp_ps[:],
                            xt_sb[e][st // 4][:, 128 * (st % 4) : 128 * (st % 4 + 1)],
                            wv_sb[e][:],
                            start=(e == 0),
                            stop=(e == EK - 1),
                        )
                    # single strided cast into the 4 head slots (skips ones col)
                    nc.vector.tensor_copy(
                        v_c[:, st, 0 : 4 * 65].rearrange("p (h d) -> p h d", h=4)[:, :, 0:64],
                        vp_ps[:].rearrange("p (h d) -> p h d", h=4),
                    )

                proj_qk(0)
                proj_qk(1)
                while rope_pend:
                    rope_tail()
                for st in range(N_SK):
                    proj_v(st)

            # ---------------- attention + output projection ----------------
            with (
                tc.tile_pool(name="attps", bufs=2, space="PSUM") as attps,
                tc.tile_pool(name="oTps", bufs=2, space="PSUM") as oTps,
                tc.tile_pool(name="expp", bufs=3) as expp,
                tc.tile_pool(name="recipp", bufs=2) as rcp,
                tc.tile_pool(name="osb", bufs=3) as osbp,
            ):
                for ch in range(N_CH):
                    cslice = slice(CH * ch, CH * (ch + 1))
                    for g in range(PAIRS):
                        hA, hB = 2 * g, 2 * g + 1
                        oTA = oTps.tile([65, CH], f32, tag="oTA")
                        oTB = oTps.tile([65, CH], f32, tag="oTB")
                        exps = []

                        def attnv(sk):
                            eAB = exps[sk]
                            first = sk == 0
                            last = sk == N_SK - 1
                            nc.tensor.matmul(
                                oTA[:],
                                v_c[:, sk, 65 * hA : 65 * hA + 65],
                                eAB[:, 0:512],
                                start=first,
                                stop=last,
                            )
                            nc.tensor.matmul(
                                oTB[:],
                                v_c[:, sk, 65 * hB : 65 * hB + 65],
                                eAB[:, 512:1024],
                                start=first,
                                stop=last,
                            )

                        for sk in range(N_SK):
                            sksl = slice(128 * sk, 128 * (sk + 1))
                            sAB = attps.tile([128, 1024], f32, tag="sAB")
                            nc.tensor.matmul(
                                sAB[:, 0:512], kT[g][:, sksl], qzlo[g][:, cslice],
                                start=True, stop=True,
                            )
                            nc.tensor.matmul(
                                sAB[:, 512:1024], kT[g][:, sksl], qzhi[g][:, cslice],
                                start=True, stop=True,
                            )
                            eAB = expp.tile([128, 1024], bf16, tag="eAB")
                            nc.scalar.activation(eAB[:], sAB[:], AF.Exp, scale=0.125)
                            exps.append(eAB)
                            if sk > 0:
                                attnv(sk - 1)
                        attnv(N_SK - 1)

                        # normalize: denominators live in row 64 of oTA/oTB.
                        # Broadcast across 64 partitions with a K=1 ones
                        # outer-product, recip, one multiply per head.
                        for head, oT in ((0, oTA), (1, oTB)):
                            # stage the denominator row to SBUF at partition 64
                            # (matches onesrow's base for the broadcast matmul)
                            oX = rcp.tile([65, CH], bf16, tag=f"oX{head}")
                            nc.vector.tensor_copy(oX[64:65, :], oT[64:65, :])
                            db = oTps.tile([64, CH], f32, tag=("oTA", "oTB")[head])
                            nc.tensor.matmul(
                                db[:], onesrow[64:65, :], oX[64:65, :],
                                start=True, stop=True,
                            )
                            rb = rcp.tile([64, CH], f32, tag=f"rb{head}")
                            nc.vector.reciprocal_approx_fast(rb[:], db[:])
                            if head == 0:
                                nc.vector.tensor_mul(
                                    att_o[g][0:64, cslice], oT[0:64, :], rb[:]
                                )
                            else:
                                aoB = rcp.tile([64, CH], bf16, tag="aoB")
                                nc.vector.tensor_mul(aoB[:], oT[0:64, :], rb[:])
                                nc.sync.dma_start(att_o[g][64:128, cslice], aoB[:])

                    # output projection for this finished sq chunk
                    for st in range(CH * ch // 128, CH * (ch + 1) // 128):
                        ssl = slice(128 * st, 128 * (st + 1))
                        op = attps.tile([128, 1024], f32, tag="sAB", name=f"op{st}")
                        for g in range(PAIRS):
                            for n in range(E // 512):
                                nsl = slice(512 * n, 512 * (n + 1))
                                nc.tensor.matmul(
                                    op[:, nsl],
                                    att_o[g][:, ssl],
                                    wo_sb[g][:, nsl],
                                    start=(g == 0),
                                    stop=(g == PAIRS - 1),
                                )
                        ot = osbp.tile([128, E], f32, tag="ot")
                        nc.vector.tensor_copy(ot[:], op[:])
                        nc.sync.dma_start(out[ssl, :], ot[:])

    nc.compile()
    return nc


def _get_program():
    if "nc" not in _BUILT:
        _BUILT["nc"] = _build_program()
    return _BUILT["nc"]


def _host_inputs(x, W_qkv, W_out):
    """Build the 8 per-core input maps (bf16)."""
    import ml_dtypes

    bf = ml_dtypes.bfloat16
    f = np.float32
    x = np.asarray(x, dtype=f)
    W_qkv = np.asarray(W_qkv, dtype=f)
    W_out = np.asarray(W_out, dtype=f)

    inv_freq = 1.0 / (ROPE_THETA ** (np.arange(0, D, 2, dtype=np.float64) / D))
    p = np.arange(128)
    freq_row = inv_freq[(p % D) // 2]  # [128]
    ang = freq_row[:, None] * np.arange(S, dtype=np.float64)[None, :]  # [128, S]
    cos_t = np.cos(ang).astype(bf)
    sign = np.where(p % 2 == 0, -1.0, 1.0)[:, None]
    sin_t = (np.sin(ang) * sign).astype(bf)

    msw = np.zeros((128, 128), dtype=bf)
    msw[p, p ^ 1] = 1.0

    maps = []
    for core in range(N_CORES):
        b, hg = divmod(core, HG)
        hs = [HPG * hg + i for i in range(HPG)]
        w_qk = np.concatenate(
            [W_qkv[:, h * D : (h + 1) * D] for h in hs]
            + [W_qkv[:, ATT + h * D : ATT + (h + 1) * D] for h in hs],
            axis=1,
        )
        w_v = np.concatenate(
            [W_qkv[:, 2 * ATT + h * D : 2 * ATT + (h + 1) * D] for h in hs], axis=1
        )
        w_o = np.concatenate([W_out[h * D : (h + 1) * D, :] for h in hs], axis=0)
        maps.append(
            {
                "xT": np.ascontiguousarray(x[b].T).astype(bf),
                "w_qk": np.ascontiguousarray(w_qk).astype(bf),
                "w_v": np.ascontiguousarray(w_v).astype(bf),
                "w_o": np.ascontiguousarray(w_o).astype(bf),
                "cos_t": cos_t,
                "sin_t": sin_t,
                "mswap": msw,
            }
        )
    return maps


def _gather(res, inputs=None):
    out = np.zeros((B, S, E), dtype=np.float32)
    for core in range(N_CORES):
        b = core // HG
        out[b] += res.results[core]["out"]
    return out


def kernel(x, W_qkv, W_out):
    from concourse.bass_utils import run_bass_kernel_spmd

    nc = _get_program()
    maps = _host_inputs(x, W_qkv, W_out)
    res = run_bass_kernel_spmd(nc, maps, core_ids=list(range(N_CORES)))
    return _gather(res)
